# revision 21
# baseline (speedup 1.0000x reference)
"""Trainium2 Bass kernel for nn_AdaptiveSNN (B=128, T=32, D=6400, H=1000, A=4).

Strategy (data-parallel over batch, 8 NeuronCores, 16 batch rows each):

  The heavy layer-1 matmul h1[b,t,:] = x[b,t,:] @ W1.T is NOT sequential in t
  (the LIF recurrence only couples the cheap elementwise state update), so per
  core we compute H1 = X_local @ W1.T as one [512, 6400] x [6400, 1024] matmul
  (H padded 1000->1024), laid out transposed: psum banks hold H1.T chunks
  [128 H, 512 cols] with col = t*16 + b (t-major, so per-step LIF slices are
  contiguous 16-element runs and layer-2 column ranges by time are contiguous).

  fp16 hi/lo x3 matmul: fp32 operands are split a = ah + al with ah = fp16(a),
  al = fp16((a - ah) * 2^12); the product needs ah*bh (psum bank HI) and
  ah*bl + al*bh (psum bank LO, uniformly scaled 2^12); al*bl (~2^-24 relative)
  is dropped.  h = HI + 2^-12 * LO then matches an fp32 matmul up to normal
  fp32 rounding (fp16 products are exact in fp32; PSUM accumulates fp32).
  W1 is pre-scaled by 256 so its lo-part stays in fp16 normal range; the LIF
  recurrence is scale-invariant, so mem1 simply runs at 256x with threshold
  256 (exact powers of two).  fp16 streams 1 cycle/row through the PE vs ~6
  effective for fp32 (measured 710ns per half-pass at N=512 even warm).

  - lhsT = W1.T tiles (host pre-transposed), rhs = X.T tiles (host
    pre-transposed), K = D on partitions, 50 k-tiles of 128.
  - m-outer loop (8 H-chunks); K=1 "ones row" matmuls fold b1 in exactly
    (hi/lo split as well).
  - LIF1 runs per group of 2 H-chunks on DVE, overlapped with the next group's
    matmuls.  Per step t: acc = beta*mem + h (scalar_tensor_tensor),
    mem = acc * keep (tensor_tensor), keep' = (mem <= thresh) (tensor_scalar).
    keep' doubles as (1 - spk), stored for layer 2.
  - Layer 2: h2 = spk1 @ W2.T + b2 = (sum(W2)+b2) - keep1 @ W2.T, computed by
    accumulating (-W2).T @ KEEP1 group tiles into one psum bank (fp32 matmuls,
    tiny) plus a K=1 ones-row with (sum(W2)+b2).  LIF2 on DVE at the end.
  - Output spk2 = 1 - keep2, written as [A, 512]; host transposes back.

  (fp32r was measured at ~1e-3 error on HW; with only ~300 output spikes a
  single threshold flip fails the rel-err gate, so only fp32-grade math is
  usable.)
"""

import sys
import types

import numpy as np

# bass_utils imports antenv.axon_hooks when BASS_TRACE is set; the module is
# absent in some images -- degrade to no tracing instead of crashing.
try:
    import antenv.axon_hooks  # noqa: F401
except ImportError:
    _m = types.ModuleType("antenv.axon_hooks")
    _m.get_axon_ntff_profile_hook = lambda: None
    _m.set_axon_ntff_profile_hook = lambda h: None
    sys.modules["antenv.axon_hooks"] = _m

import concourse.bass as bass
import concourse.tile as tile
from concourse import bacc, mybir
from concourse.bass_utils import run_bass_kernel_spmd

F32 = mybir.dt.float32
F16 = mybir.dt.float16
OP = mybir.AluOpType

NCORES = 8
B, T, D, H, A = 128, 32, 6400, 1000, 4
BL = B // NCORES            # 16 local batch
COLS = BL * T               # 512 matmul columns, col = t*16 + b (t-major)
KT = D // 128               # 50 k tiles
HP = 1024                   # padded H
M = HP // 128               # 8 H-chunks
BETA = 1.0 - 0.01

WSCALE = 256.0              # W1 pre-scale (exact power of 2)
LSCALE = 4096.0             # lo-part scale 2^12

# FP16X3 True: hi/lo fp16 3-pass matmul.  False: plain fp32 matmul.
FP16X3 = True

XCH = 5                     # x DMA chunks (10 k-tiles each)
XKT = KT // XCH
W1H = 2                     # w1 DMA halves per m-chunk (25 k-tiles each)
W1KT = KT // W1H

_CACHE = {}


def _lif_steps(nc, memv, accv, h_at, k_at, thresh, t_range=None):
    """Emit the LIF recurrence for timesteps in t_range (default all).

    memv/accv: [p, ..., b] fp32 SBUF views; h_at(t)/k_at(t) return the
    per-step views.  keep column t holds (mem_t <= thresh) = 1 - spk_t.
    """
    for t in (t_range if t_range is not None else range(T)):
        if t == 0:
            # mem=0, keep=1: mem <- h_0  (beta*0 + h)
            nc.vector.scalar_tensor_tensor(
                out=memv, in0=memv, scalar=BETA,
                in1=h_at(0), op0=OP.mult, op1=OP.add)
        else:
            nc.vector.scalar_tensor_tensor(
                out=accv, in0=memv, scalar=BETA,
                in1=h_at(t), op0=OP.mult, op1=OP.add)
            nc.vector.tensor_tensor(
                out=memv, in0=accv, in1=k_at(t - 1), op=OP.mult)
        nc.vector.tensor_scalar(
            out=k_at(t), in0=memv, scalar1=thresh,
            scalar2=None, op0=OP.is_le)


def build(with_b1=True):
    nc = bacc.Bacc("TRN2", target_bir_lowering=False, debug=False,
                   num_devices=NCORES)

    MMDT = F16 if FP16X3 else F32
    THR1 = 1.0 * WSCALE if FP16X3 else 1.0

    # host layouts (see kernel() for the exact host-side packing):
    #   xh/xl [128(p), KT, COLS]      x.T tiles, col = t*16+b, hi/lo fp16
    #   w1h/w1l [M, 128(p), KT, 128]  (256*W1).T tiles, hi/lo fp16
    #   b1h/b1l [1, HP]               256*b1 hi/lo rows
    #   w2n  [128(p), M*A]            w2n[p, m*4+a] = -W2p[a, m*128+p] (fp32)
    #   s2b2 [1, A]                   sum(W2p, axis=1) + b2 (fp32)
    xh_e = nc.declare_dram_parameter("xh", [128, KT, COLS], MMDT, isOutput=False)
    w1h_e = nc.declare_dram_parameter("w1h", [M, 128, KT, 128], MMDT, isOutput=False)
    b1h_e = nc.declare_dram_parameter("b1hl", [1, (2 * HP if FP16X3 else HP)],
                                      MMDT, isOutput=False)
    if FP16X3:
        xl_e = nc.declare_dram_parameter("xl", [128, KT, COLS], F16, isOutput=False)
        w1l_e = nc.declare_dram_parameter("w1l", [M, 128, KT, 128], F16, isOutput=False)
    if FP16X3:
        # negated W2 in hi/lo fp16 (lo scaled 2^12): [hi | lo] blocks
        w2_e = nc.declare_dram_parameter("w2x", [128, 2 * M * A], F16,
                                         isOutput=False)
        # [s2h | s2l' | -s2h] rows for the dual-region opener
        s2_e = nc.declare_dram_parameter("s2x", [1, 3 * A], F16, isOutput=False)
    else:
        w2_e = nc.declare_dram_parameter("w2n", [128, M * A], F32, isOutput=False)
        s2_e = nc.declare_dram_parameter("s2b2", [1, A], F32, isOutput=False)
    out_e = nc.declare_dram_parameter("out", [A, COLS], F32, isOutput=True)

    with tile.TileContext(nc) as tc:
        with (
            tc.tile_pool(name="const", bufs=1) as cpool,
            tc.tile_pool(name="xsb", bufs=(2 * XCH if FP16X3 else XCH)) as xpool,
            tc.tile_pool(name="w1", bufs=(6 if FP16X3 else 4)) as wpool,
            tc.tile_pool(name="h1g", bufs=2) as hpool,
            tc.tile_pool(name="keep", bufs=2) as kpool,
            tc.tile_pool(name="scratch", bufs=2) as spool,
            tc.tile_pool(name="ps1", bufs=(6 if FP16X3 else 7), space="PSUM") as ps1,
            tc.tile_pool(name="ps2", bufs=1, space="PSUM") as ps2,
        ):
            # Small constants + x go on the Scalar HWDGE queue; W1 streams on
            # the Sync HWDGE queue.  Two independent FIFOs -> W1's first tiles
            # aren't stuck behind 13MB of x (measured 38us PE stall).
            ones = cpool.tile([1, COLS], MMDT)
            nc.vector.memset(ones, 1.0)
            ones32 = cpool.tile([1, COLS], F32)
            nc.vector.memset(ones32, 1.0)
            b1hl = cpool.tile([1, (2 * HP if FP16X3 else HP)], MMDT)
            nc.scalar.dma_start(out=b1hl, in_=b1h_e.ap())
            b1h = b1hl[:, :HP]
            if FP16X3:
                b1l = b1hl[:, HP:]

            mem1 = cpool.tile([128, M * BL], F32)
            nc.vector.memset(mem1, 0.0)
            mem1v = mem1.rearrange("p (m b) -> p m b", m=M)
            mem2 = cpool.tile([A, BL], F32)
            nc.vector.memset(mem2, 0.0)
            keep2 = cpool.tile([A, COLS], F32)
            k2v = keep2.rearrange("p (t b) -> p b t", t=T)
            h2sb = cpool.tile([A, COLS], F32)
            h2v = h2sb.rearrange("p (t b) -> p b t", t=T)
            spk2 = cpool.tile([A, COLS], F32)
            acc2 = cpool.tile([A, BL], F32)

            # x load; chunk 0 split finer so the PE can start sooner
            xparams = [xh_e, xl_e] if FP16X3 else [xh_e]
            xtiles = [[] for _ in xparams]
            for xc in range(XCH):
                xts = [xpool.tile([128, XKT * COLS], MMDT, tag="x", name=f"x{xi}")
                       for xi in range(len(xparams))]
                # hi/lo pieces interleaved so the lo stream is never a full
                # chunk behind the hi stream the PE is consuming
                npieces = 2 if xc == 0 else 1
                edges = [xc * XKT + (XKT * p) // npieces
                         for p in range(npieces + 1)]
                for p in range(npieces):
                    k0, k1 = edges[p], edges[p + 1]
                    o0 = (k0 - xc * XKT) * COLS
                    o1 = (k1 - xc * XKT) * COLS
                    for xi, xe in enumerate(xparams):
                        nc.scalar.dma_start(
                            out=xts[xi][:, o0:o1], in_=xe.ap()[:, k0:k1, :])
                for xi in range(len(xparams)):
                    xtiles[xi].append(xts[xi])

            def x_rhs(xi, k):
                xt = xtiles[xi][k // XKT]
                o = (k % XKT) * COLS
                return xt[:, o:o + COLS]

            # w2/s2 are not needed until the first group finishes
            W2DT = F16 if FP16X3 else F32
            w2sb = cpool.tile([128, (2 * M * A if FP16X3 else M * A)], W2DT)
            nc.scalar.dma_start(out=w2sb, in_=w2_e.ap())
            s2sb = cpool.tile([1, (3 * A if FP16X3 else A)], W2DT)
            nc.scalar.dma_start(out=s2sb, in_=s2_e.ap())
            # One psum bank per time-half.  With fp16 W2 the bank holds two
            # regions: hi sums in cols [0,CH), lo sums (2^12-scaled) in
            # [CH,2CH); a single full-width start=True opener avoids the
            # illegal interleaved-starts-on-one-bank pattern.
            psum2h = [ps2.tile([A, COLS], F32, name=f"p2_{h}", tag=f"p2{h}")
                      for h in range(2)]

            wparams = [w1h_e, w1l_e] if FP16X3 else [w1h_e]
            TH = T // 2
            CH = COLS // 2          # column half, t-major: cols [0,CH) = t<TH

            def stream_w1(ms, hf, finely=False):
                """Stream this k-half of W1 for the chunks in ms, pieces
                interleaved across (chunk, hi/lo) so consumers stay in
                lockstep.  Returns {(chunk_idx, dtype_idx): tile}."""
                tiles = {}
                for i in range(len(ms)):
                    for wi in range(len(wparams)):
                        tiles[(i, wi)] = wpool.tile(
                            [128, W1KT * 128], MMDT, tag="w1", name="w1t")
                nq = 5 if finely else 1
                step = W1KT // nq
                for q in range(nq):
                    for i, m in enumerate(ms):
                        for wi, we in enumerate(wparams):
                            nc.sync.dma_start(
                                out=tiles[(i, wi)][:, q * step * 128:
                                                   (q + 1) * step * 128],
                                in_=we.ap()[m, :, hf * W1KT + q * step:
                                            hf * W1KT + (q + 1) * step, :])
                return tiles

            def k_sweep(ms, phs, pls, cs, finely=False):
                """Bias + 50 k-tile matmuls for the chunks in ms over column
                slice cs, k-interleaved across chunks (spreads the DMA demand
                of the first group over twice the PE time)."""
                ncols = cs.stop - cs.start
                if with_b1:
                    for i, m in enumerate(ms):
                        nc.tensor.matmul(
                            phs[i], lhsT=b1h[:, m * 128:(m + 1) * 128],
                            rhs=ones[:, :ncols], start=True, stop=False)
                        if FP16X3:
                            nc.tensor.matmul(
                                pls[i], lhsT=b1l[:, m * 128:(m + 1) * 128],
                                rhs=ones[:, :ncols], start=True, stop=False)
                for hf in range(W1H):
                    wts = stream_w1(ms, hf, finely=(finely and hf == 0))
                    for kk in range(W1KT):
                        k = hf * W1KT + kk
                        start = (not with_b1) and k == 0
                        last = (k == KT - 1)
                        sl = slice(kk * 128, (kk + 1) * 128)
                        # hi*hi -> HI bank; hi*lo + lo*hi -> LO bank
                        for i in range(len(ms)):
                            nc.tensor.matmul(
                                phs[i], lhsT=wts[(i, 0)][:, sl],
                                rhs=x_rhs(0, k)[:, cs],
                                start=start, stop=last)
                            if FP16X3:
                                nc.tensor.matmul(
                                    pls[i], lhsT=wts[(i, 0)][:, sl],
                                    rhs=x_rhs(1, k)[:, cs],
                                    start=start, stop=False)
                                nc.tensor.matmul(
                                    pls[i], lhsT=wts[(i, 1)][:, sl],
                                    rhs=x_rhs(0, k)[:, cs],
                                    start=False, stop=last)

            def evac(hslc, ph, pl):
                # h = HI + 2^-12 * LO  (h stays at 256*h1 scale)
                nc.vector.tensor_copy(hslc, ph)
                if FP16X3:
                    nc.vector.scalar_tensor_tensor(
                        out=hslc, in0=pl, scalar=1.0 / LSCALE, in1=hslc,
                        op0=OP.mult, op1=OP.add)

            mm2_open = [False, False]

            def mm2(m, keep_ap, half, stop=False):
                """Accumulate chunk m of h2 = (s2+b2) - keep @ W2.T into
                psum2h[half] (fp16 path: hi+lo regions of one bank)."""
                p2 = psum2h[half]
                if FP16X3:
                    if not mm2_open[half]:
                        # full-width opener with s2h, then patch the lo
                        # region to s2l' via (+s2l', -s2h) exact fp16 rows
                        nc.tensor.matmul(p2, lhsT=s2sb[:, 0:A], rhs=ones,
                                         start=True, stop=False,
                                         skip_group_check=True)
                        nc.tensor.matmul(p2[:, CH:], lhsT=s2sb[:, A:2 * A],
                                         rhs=ones[:, :CH], start=False,
                                         stop=False, skip_group_check=True)
                        nc.tensor.matmul(p2[:, CH:], lhsT=s2sb[:, 2 * A:3 * A],
                                         rhs=ones[:, :CH], start=False,
                                         stop=False, skip_group_check=True)
                        mm2_open[half] = True
                    nc.tensor.matmul(
                        p2[:, :CH], lhsT=w2sb[:, m * A:(m + 1) * A],
                        rhs=keep_ap, start=False, stop=False,
                        skip_group_check=True)
                    nc.tensor.matmul(
                        p2[:, CH:], lhsT=w2sb[:, (M + m) * A:(M + m + 1) * A],
                        rhs=keep_ap, start=False, stop=stop,
                        skip_group_check=True)
                else:
                    if not mm2_open[half]:
                        nc.tensor.matmul(p2[:, :CH], lhsT=s2sb,
                                         rhs=ones32[:, :CH],
                                         start=True, stop=False,
                                         skip_group_check=True)
                        mm2_open[half] = True
                    nc.tensor.matmul(
                        p2[:, :CH], lhsT=w2sb[:, m * A:(m + 1) * A],
                        rhs=keep_ap, start=False, stop=stop,
                        skip_group_check=True)

            GROUPS = [(0, 1), (2, 3), (4, 5), (6,)]
            THR = THR1
            for gms in GROUPS:
                nch = len(gms)
                h1g = hpool.tile([128, nch * COLS], F32, tag="h1g")
                phs = [ps1.tile([128, COLS], F32, tag="ps1", name="ph")
                       for _ in gms]
                pls = [ps1.tile([128, COLS], F32, tag="ps1", name="pl")
                       for _ in gms] if FP16X3 else [None] * nch
                k_sweep(gms, phs, pls, slice(0, COLS), finely=(gms[0] == 0))
                for c, m in enumerate(gms):
                    evac(h1g[:, c * COLS:(c + 1) * COLS], phs[c], pls[c])
                h4 = h1g.rearrange("p (c t b) -> p c b t", c=nch, t=T)
                keepg = kpool.tile([128, nch * COLS], MMDT, tag="keep")
                k4 = keepg.rearrange("p (c t b) -> p c b t", c=nch, t=T)
                memv = mem1v[:, gms[0]:gms[0] + nch, :]
                accg = spool.tile([128, nch * BL], F32, tag="acc")
                accv = accg.rearrange("p (c b) -> p c b", c=nch)
                _lif_steps(nc, memv, accv,
                           lambda t: h4[..., t], lambda t: k4[..., t], THR)
                for c, m in enumerate(gms):
                    for half in range(2):
                        mm2(m, keepg[:, c * COLS + half * CH:
                                     c * COLS + (half + 1) * CH], half)

            # m = 7 runs in two column (time) phases: while the PE sweeps
            # phase B (t >= 16), the DVE runs LIF1(m7) + LIF2 for t < 16.
            m = M - 1
            h1g7 = hpool.tile([128, COLS], F32, tag="h1g")
            keep7 = kpool.tile([128, COLS], MMDT, tag="keep")
            mem7 = mem1v[:, m, :]
            acc7v = spool.tile([128, BL], F32, tag="acc", name="acc7")
            for half in range(2):
                cs = slice(half * CH, (half + 1) * CH)
                ph = ps1.tile([128, CH], F32, tag="ps1", name="ph7")
                pl = (ps1.tile([128, CH], F32, tag="ps1", name="pl7")
                      if FP16X3 else None)
                k_sweep([m], [ph], [pl], cs)
                evac(h1g7[:, cs], ph, pl)
                _lif_steps(nc, mem7, acc7v,
                           lambda t: h1g7[:, t * BL:(t + 1) * BL],
                           lambda t: keep7[:, t * BL:(t + 1) * BL], THR,
                           t_range=range(half * TH, (half + 1) * TH))
                mm2(m, keep7[:, cs], half, stop=True)
                # layer-2 for this time half; spk2 = 1 - keep2; half A's
                # output DMA hides under phase B's matmul sweep
                nc.vector.tensor_copy(h2sb[:, cs], psum2h[half][:, :CH])
                if FP16X3:
                    nc.vector.scalar_tensor_tensor(
                        out=h2sb[:, cs], in0=psum2h[half][:, CH:],
                        scalar=1.0 / LSCALE, in1=h2sb[:, cs],
                        op0=OP.mult, op1=OP.add)
                _lif_steps(nc, mem2, acc2,
                           lambda t: h2sb[:, t * BL:(t + 1) * BL],
                           lambda t: keep2[:, t * BL:(t + 1) * BL], 1.0,
                           t_range=range(half * TH, (half + 1) * TH))
                nc.vector.tensor_scalar(
                    out=spk2[:, cs], in0=keep2[:, cs], scalar1=-1.0,
                    scalar2=1.0, op0=OP.mult, op1=OP.add)
                nc.sync.dma_start(out=out_e.ap()[:, cs], in_=spk2[:, cs])

    nc.compile()
    return nc


def _split16(a):
    """fp32 array -> (hi, lo) fp16 with lo scaled by 2^12."""
    hi = a.astype(np.float16)
    lo = ((a - hi.astype(np.float32)) * LSCALE).astype(np.float16)
    return hi, lo


def _prep_shared(W1, b1, W2, b2):
    W1p = np.zeros((HP, D), np.float32)
    W1p[:H] = W1
    b1p = np.zeros((1, HP), np.float32)
    b1p[0, :H] = b1
    if FP16X3:
        W1p *= WSCALE
        b1p = b1p * WSCALE
    # w1T[m,p,k,j] = W1p[m*128+j, k*128+p]
    w1T = np.ascontiguousarray(
        W1p.reshape(M, 128, KT, 128).transpose(0, 3, 2, 1))
    W2p = np.zeros((A, HP), np.float32)
    W2p[:, :H] = W2
    # w2n[p, m*4+a] = -W2p[a, m*128+p]
    w2n = np.ascontiguousarray((-W2p).reshape(A, M, 128).transpose(2, 1, 0)
                               .reshape(128, M * A))
    s2b2 = (W2p.sum(axis=1, dtype=np.float32) + b2).reshape(1, A)
    s2b2 = np.ascontiguousarray(s2b2.astype(np.float32))
    if FP16X3:
        shared = {}
        w2h, w2l = _split16(w2n)
        shared["w2x"] = np.ascontiguousarray(
            np.concatenate([w2h, w2l], axis=1))
        s2h, s2l = _split16(s2b2)
        shared["s2x"] = np.ascontiguousarray(
            np.concatenate([s2h, s2l, -s2h], axis=1))
        shared["w1h"], shared["w1l"] = _split16(w1T)
        bh, bl = _split16(b1p)
        shared["b1hl"] = np.concatenate([bh, bl], axis=1)
    else:
        shared = {"w2n": w2n, "s2b2": s2b2}
        shared["w1h"] = w1T
        shared["b1hl"] = b1p
    return shared


def _prep_x(x, c):
    # rows t-major: row = t*16 + b
    xs = np.ascontiguousarray(
        x[c * BL:(c + 1) * BL].transpose(1, 0, 2)).reshape(COLS, D)
    xT = np.ascontiguousarray(xs.T)                    # [D, COLS]
    # [128(p), KT, COLS]: xT3[p,k,c] = xT[k*128+p, c]
    xT3 = np.ascontiguousarray(xT.reshape(KT, 128, COLS).transpose(1, 0, 2))
    if FP16X3:
        hi, lo = _split16(xT3)
        return {"xh": hi, "xl": lo}
    return {"xh": xT3}


def kernel(x, W1, b1, W2, b2, _want_results=False):
    x = np.ascontiguousarray(np.asarray(x), np.float32)
    W1 = np.asarray(W1, np.float32)
    b1 = np.asarray(b1, np.float32)
    W2 = np.asarray(W2, np.float32)
    b2 = np.asarray(b2, np.float32)

    with_b1 = bool(np.any(b1))
    key = ("nc", with_b1)
    if key not in _CACHE:
        _CACHE[key] = build(with_b1=with_b1)
    nc = _CACHE[key]

    shared = _prep_shared(W1, b1, W2, b2)
    in_maps = []
    for c in range(NCORES):
        m = dict(shared)
        m.update(_prep_x(x, c))
        in_maps.append(m)

    res = run_bass_kernel_spmd(nc, in_maps, core_ids=list(range(NCORES)))

    out = np.empty((B, T, A), np.float32)
    for c in range(NCORES):
        o = res.results[c]["out"]                      # [A, COLS], col=t*16+b
        out[c * BL:(c + 1) * BL] = o.T.reshape(T, BL, A).transpose(1, 0, 2)
    if _want_results:
        return out, res
    return out
